# revision 1
# baseline (speedup 1.0000x reference)
"""Trainium2 Bass kernel for CategoricalEntropyRegLoss.

Math: both loss terms factor so the [B,B] pairwise matrices are never built.

  feat_dists = sq_j + sq_k - 2 fn_j.fn_k            (rank FD+2)
  target_dists = (E_j - P_j.LQ_k) / D               (rank DC+1)
  S = sum_{jk} m_j m_k feat_dists * target_dists    (diag is exactly 0)
    = [ se*M + a*e - 2 Fe.F - Psq.L - Pbar.Lsq + 2 <U,V> ] / D
  tightness*M = a - sum_s ||seg_sum_s||^2 / max(cnt_s,1)

Everything needed is one matmul per core:
  out[1154, 258] = ext_seg^T @ ext_feat
  ext_seg  = [ onehot(code) | LQ | P | 1 | E ]      (B x 1154)
  ext_feat = [ m*fn | m | m*sq ]                    (B x 258)
followed by a single 8-core AllReduce (fp32) of the [1154,258] partials
and a cheap redundant epilogue on every core.

All matmul operands and the AllReduce payload are fp32 (the total output
has ~7x cancellation amplification, so bf16 operands would cost ~1 digit
of the margin). Measured end-to-end rel err ~2.5e-6.
"""

import numpy as np

B = 4096
FD = 256
C = 32
D = 2
NSEG = C ** D          # 1024
NCORES = 8
RB = B // NCORES       # 512 rows per core
KT = RB // 128         # 4 k-chunks of 128 rows
EF = FD + 2            # 258: [mfn | m | m*sq]
ES = NSEG + 2 * D * C + 2   # 1154: [onehot | LQ | P | ones | E]
PCOL = NSEG + D * C    # 1088: start of P block
LCOL = NSEG            # 1024: start of LQ block
ONES_COL = NSEG + 2 * D * C      # 1152
E_COL = ONES_COL + 1             # 1153
NMT = (ES + 127) // 128          # 10 m-tiles (last has 2 rows)

_compiled = {}


def _build_bass():
    from contextlib import ExitStack
    import concourse.bass as bass
    import concourse.bacc as bacc
    import concourse.tile as tile
    from concourse import mybir

    from concourse.tile import add_dep_helper

    f32 = mybir.dt.float32
    bf16 = mybir.dt.bfloat16
    Alu = mybir.AluOpType
    Act = mybir.ActivationFunctionType
    Ax = mybir.AxisListType

    nc = bacc.Bacc(num_devices=NCORES)

    feat = nc.dram_tensor("features", [RB, FD], f32, kind="ExternalInput")
    targ = nc.dram_tensor("targets", [RB, D * C], f32, kind="ExternalInput")
    maskf = nc.dram_tensor("maskf", [RB, 1], f32, kind="ExternalInput")
    outd = nc.dram_tensor("out", [8], f32, kind="ExternalOutput")

    with ExitStack() as ctx:
        tc = ctx.enter_context(tile.TileContext(nc))
        consts = ctx.enter_context(tc.tile_pool(name="consts", bufs=1))
        work = ctx.enter_context(tc.tile_pool(name="work", bufs=1))
        keep = ctx.enter_context(tc.tile_pool(name="keep", bufs=1))
        res_pool = ctx.enter_context(tc.tile_pool(name="res", bufs=1))
        psum = ctx.enter_context(tc.tile_pool(name="psum", bufs=1, space="PSUM"))
        dram = ctx.enter_context(tc.tile_pool(name="dram", bufs=1, space="DRAM"))

        # ---------------- constants ----------------
        ones128 = consts.tile([128, 1], f32)
        nc.vector.memset(ones128[:], 1.0)

        # partition-major AllReduce buffer: [p, mt, f]. Elementwise AR is
        # layout-agnostic, and this makes every store/load contiguous per
        # partition (the [row, f] layout's 1KB descriptor lines cost ~5us
        # per 0.26MB strided DMA). Slot (p, 9, :) is zero-padded for p>=2.
        inbounce = dram.tile([128, NMT, EF], f32, name="inbounce")
        outbounce = dram.tile([128, NMT, EF], f32, name="outbounce",
                              addr_space="Shared")

        # ---- batched input loads spread over two queues ----
        tbig = keep.tile([128, KT, D * C], f32, name="tbig")
        nc.scalar.dma_start(
            out=tbig[:], in_=targ[:, :].rearrange("(a p) f -> p a f", p=128))
        mkbig = keep.tile([128, KT, 1], f32, name="mkbig")
        nc.scalar.dma_start(
            out=mkbig[:], in_=maskf[:, :].rearrange("(a p) f -> p a f", p=128))
        # two tiles (not halves of one) so chunk reads only wait their own DMA
        xbig0 = keep.tile([128, 2, FD], f32, name="xbig0")
        nc.sync.dma_start(
            out=xbig0[:],
            in_=feat[0:256, :].rearrange("(a p) f -> p a f", p=128))
        xbig1 = keep.tile([128, 2, FD], f32, name="xbig1")
        nc.gpsimd.dma_start(
            out=xbig1[:],
            in_=feat[256:512, :].rearrange("(a p) f -> p a f", p=128))

        def xchunk(kc):
            return xbig0[:, kc, :] if kc < 2 else xbig1[:, kc - 2, :]

        # iotas after the gpsimd input DMA trigger (not needed until ~15us)
        iota1024 = consts.tile([128, NSEG], f32)
        nc.gpsimd.iota(iota1024[:], [[1, NSEG]], channel_multiplier=0,
                       allow_small_or_imprecise_dtypes=True)
        # biota[j] = 32 - j  (for first-argmax via reduce_max)
        biota = consts.tile([128, C], f32)
        nc.gpsimd.iota(biota[:], [[-1, C]], base=C, channel_multiplier=0,
                       allow_small_or_imprecise_dtypes=True)


        # fp32 throughout the matmul operands: the total output suffers ~7x
        # cancellation amplification, so bf16 inputs (rel err ~1.6e-5) risk a
        # tight scale-relative threshold; fp32 lands at ~2.5e-6 for ~5us more.
        NST = 2 * D * C + 2   # 130 stats columns: [lq | p | ones | E]
        es_oh = [keep.tile([128, NSEG], f32, name=f"esoh_{kc}")
                 for kc in range(KT)]
        es_st = [keep.tile([128, NST], f32, name=f"esst_{kc}")
                 for kc in range(KT)]
        ef_f32 = [keep.tile([128, EF], f32, name=f"eff_{kc}")
                  for kc in range(KT)]
        ef_b16 = ef_f32

        # ---- ACT phase 1: row sum-of-squares (Square table loads once) ----
        sqpack = keep.tile([128, KT], f32, name="sqpack")
        scrsq = keep.tile([128, FD], f32, name="scrsq")
        act_chain = []
        for kc in range(KT):
            act_chain.append(nc.scalar.activation(
                out=scrsq[:], in_=xchunk(kc), func=Act.Square,
                accum_out=sqpack[:, kc:kc + 1]))
        # ---- ACT phase 2: one Sqrt for all chunks ----
        normpack = keep.tile([128, KT], f32, name="normpack")
        act_chain.append(nc.scalar.sqrt(normpack[:], sqpack[:]))
        nc.vector.tensor_scalar_max(out=normpack[:], in0=normpack[:],
                                    scalar1=1e-12)
        invpack = keep.tile([128, KT], f32, name="invpack")
        nc.vector.reciprocal(invpack[:], normpack[:])
        # minv = m * inv  (fold mask into the normalization scale)
        minvpack = keep.tile([128, KT], f32, name="minvpack")
        nc.vector.tensor_tensor(out=minvpack[:], in0=invpack[:],
                                in1=mkbig[:, :, 0], op=Alu.mult)

        # ---- targets chains (DVE) + Ln (ACT phase 3) ----
        # es_st columns: [0:64 lq | 64:128 p | 128 ones | 129 E]
        # chunk-batched front: one add / one reduce / one reciprocal
        t1big = keep.tile([128, KT, D * C], f32, name="t1big")
        nc.vector.tensor_scalar_add(out=t1big[:], in0=tbig[:], scalar1=1e-10)
        invsb = keep.tile([128, KT * D], f32, name="invsb")
        nc.vector.reduce_sum(
            out=invsb[:],
            in_=t1big[:].rearrange("p a (d c) -> p (a d) c", c=C),
            axis=Ax.X)
        nc.vector.reciprocal(invsb[:], invsb[:])
        ln_acts = []
        for kc in range(KT):
            st_t = es_st[kc]
            pt = st_t[:, D * C:2 * D * C]
            lqt = st_t[:, 0:D * C]
            for d_ in range(D):
                nc.vector.tensor_scalar_mul(
                    out=pt[:, C * d_:C * (d_ + 1)],
                    in0=t1big[:, kc, C * d_:C * (d_ + 1)],
                    scalar1=invsb[:, kc * D + d_:kc * D + d_ + 1])
            ln_acts.append(nc.scalar.activation(out=lqt, in_=pt,
                                                func=Act.Ln))

            # ---- first-argmax per dim, then code = cls0 + 32*cls1 ----
            cls = work.tile([128, D], f32, name=f"cls_{kc}", tag=f"cl_{kc}")
            for d_ in range(D):
                pch = pt[:, C * d_:C * (d_ + 1)]
                mx = work.tile([128, 1], f32, name=f"mx_{kc}_{d_}",
                               tag=f"mx_{kc}_{d_}")
                nc.vector.reduce_max(out=mx[:], in_=pch, axis=Ax.X)
                cand = work.tile([128, C], f32, name=f"cand_{kc}_{d_}",
                                 tag=f"cd_{kc}_{d_}")
                # (p == max) * (32 - idx); reduce_max -> 32 - first_argmax
                nc.vector.scalar_tensor_tensor(
                    out=cand[:], in0=pch, scalar=mx[:], in1=biota[:],
                    op0=Alu.is_equal, op1=Alu.mult)
                mq = work.tile([128, 1], f32, name=f"mq_{kc}_{d_}",
                               tag=f"mq_{kc}_{d_}")
                nc.vector.reduce_max(out=mq[:], in_=cand[:], axis=Ax.X)
                nc.vector.tensor_scalar(
                    out=cls[:, d_:d_ + 1], in0=mq[:], scalar1=-1.0,
                    scalar2=float(C), op0=Alu.mult, op1=Alu.add)
            code = work.tile([128, 1], f32, name=f"code_{kc}", tag=f"co_{kc}")
            nc.vector.tensor_scalar(
                out=code[:], in0=cls[:, 1:2], scalar1=float(C),
                scalar2=cls[:, 0:1], op0=Alu.mult, op1=Alu.add)
            # ---- one-hot (DVE; gpsimd runs this 20x slower AND port-starves
            # concurrent DVE ops — measured 15.6us per tile there) ----
            nc.vector.tensor_scalar(
                out=es_oh[kc][:], in0=iota1024[:], scalar1=code[:],
                scalar2=None, op0=Alu.is_equal)

        # ---- ext_feat = [x*(m*inv) | m | sq0*inv*minv] (ACT phase 4) ----
        copy_acts = []
        for kc in range(KT):
            ef_t = ef_f32[kc]
            copy_acts.append(nc.scalar.activation(
                out=ef_t[:, 0:FD], in_=xchunk(kc), func=Act.Copy,
                scale=minvpack[:, kc:kc + 1]))
            nc.vector.tensor_copy(out=ef_t[:, FD:FD + 1], in_=mkbig[:, kc, :])
            nc.vector.tensor_scalar(out=ef_t[:, FD + 1:FD + 2],
                                    in0=sqpack[:, kc:kc + 1],
                                    scalar1=invpack[:, kc:kc + 1],
                                    scalar2=minvpack[:, kc:kc + 1],
                                    op0=Alu.mult, op1=Alu.mult)

        # E / ones columns, deferred: only the last two m-tiles need them
        for kc in range(KT):
            st_t = es_st[kc]
            scr64 = work.tile([128, D * C], f32, name=f"scr64_{kc}",
                              tag=f"s64_{kc}")
            nc.vector.tensor_tensor(out=scr64[:],
                                    in0=st_t[:, D * C:2 * D * C],
                                    in1=st_t[:, 0:D * C], op=Alu.mult)
            nc.vector.reduce_sum(out=st_t[:, NST - 1:NST], in_=scr64[:],
                                 axis=Ax.X)
            nc.vector.memset(st_t[:, NST - 2:NST - 1], 1.0)

        # keep ACT ops grouped by function (avoid act-table reload thrash);
        # table-less Copies run before the Lns so ef is ready sooner
        act_chain = act_chain + copy_acts + ln_acts
        for a, b in zip(act_chain[1:], act_chain[:-1]):
            add_dep_helper(a.ins, b.ins, sync=False,
                           reason="act table grouping")

        # ---------------- the one big matmul ----------------
        # separate result tiles per store so no DMA reads a tile that later
        # copies write (a shared tile serializes copies on false WAR deps)
        resa = keep.tile([128, 4, EF], f32, name="resa")
        resb = keep.tile([128, 4, EF], f32, name="resb")
        resc = keep.tile([128, 2, EF], f32, name="resc")
        nc.vector.memset(resc[:], 0.0)
        for mt in range(NMT):
            mlo = mt * 128
            msz = min(128, ES - mlo)
            ps = psum.tile([msz, EF], f32, name=f"ps_{mt}", tag=f"ps_{mt % 7}")
            for kc in range(KT):
                if mt < 8:
                    lhsT = es_oh[kc][:, mlo:mlo + msz]
                    rhs = ef_b16[kc][:]
                else:
                    lhsT = es_st[kc][:, mlo - NSEG:mlo - NSEG + msz]
                    rhs = ef_f32[kc][:]
                nc.tensor.matmul(out=ps[:], lhsT=lhsT, rhs=rhs,
                                 start=(kc == 0), stop=(kc == KT - 1))
            if mt < 4:
                nc.vector.tensor_copy(out=resa[:, mt, :], in_=ps[:])
            elif mt < 8:
                nc.vector.tensor_copy(out=resb[:, mt - 4, :], in_=ps[:])
            else:
                nc.vector.tensor_copy(out=resc[0:msz, mt - 8, :], in_=ps[:])
            if mt == 3:
                nc.sync.dma_start(out=inbounce[:, 0:4, :], in_=resa[:])
            elif mt == 7:
                nc.gpsimd.dma_start(out=inbounce[:, 4:8, :], in_=resb[:])
            elif mt == 9:
                nc.scalar.dma_start(out=inbounce[:, 8:10, :], in_=resc[:])

        # ---------------- single AllReduce ----------------
        nc.gpsimd.collective_compute(
            "AllReduce", mybir.AluOpType.add,
            replica_groups=[list(range(NCORES))],
            ins=[inbounce.opt()], outs=[outbounce.opt()])

        # ---------------- epilogue (redundant on every core) ----------------
        # contiguous loads of the segment slots, split over two queues so
        # the DVE squares on half 0 overlap half 1's transfer
        big0 = keep.tile([128, 4, EF], f32, name="big0")
        nc.sync.dma_start(out=big0[:], in_=outbounce[:, 0:4, :])
        big1 = keep.tile([128, 4, EF], f32, name="big1")
        nc.scalar.dma_start(out=big1[:], in_=outbounce[:, 4:8, :])
        last2 = keep.tile([1, EF], f32, name="last2")
        nc.scalar.dma_start(out=last2[:], in_=outbounce[0:1, 9, :])
        r1 = keep.tile([1, EF], f32, name="r1")
        nc.scalar.dma_start(out=r1[:], in_=outbounce[1:2, 9, :])

        Z = keep.tile([128, 8], f32, name="Z")
        nc.vector.memset(Z[:], 0.0)
        nrmp = keep.tile([128, 8], f32, name="nrmp")
        cdp = keep.tile([128, 8], f32, name="cdp")
        # squares on DVE (ACT<->DVE ping-pong here costs ~4us otherwise)
        scrA = keep.tile([128, 4, FD], f32, name="scrA")
        nc.vector.tensor_tensor(out=scrA[:], in0=big0[:, :, 0:FD],
                                in1=big0[:, :, 0:FD], op=Alu.mult)
        nc.vector.reduce_sum(out=nrmp[:, 0:4], in_=scrA[:], axis=Ax.X)
        scrB = keep.tile([128, 4, FD], f32, name="scrB")
        nc.vector.tensor_tensor(out=scrB[:], in0=big1[:, :, 0:FD],
                                in1=big1[:, :, 0:FD], op=Alu.mult)
        red_b = nc.vector.reduce_sum(out=nrmp[:, 4:8], in_=scrB[:], axis=Ax.X)
        nc.vector.tensor_scalar_max(out=cdp[:, 0:4], in0=big0[:, :, FD],
                                    scalar1=1.0)
        nc.vector.tensor_scalar_max(out=cdp[:, 4:8], in0=big1[:, :, FD],
                                    scalar1=1.0)
        rcdp = keep.tile([128, 8], f32, name="rcdp")
        nc.vector.reciprocal(rcdp[:], cdp[:])
        termp = keep.tile([128, 8], f32, name="termp")
        nc.vector.tensor_tensor(out=termp[:], in0=nrmp[:], in1=rcdp[:],
                                op=Alu.mult)
        nc.vector.reduce_sum(out=Z[:, 0:1], in_=termp[:], axis=Ax.X)

        # stats m-tile 8: partitions 0:64 = U^T rows, 64:128 = V^T rows
        ut = keep.tile([64, EF], f32, name="ut")
        nc.sync.dma_start(out=ut[:], in_=outbounce[0:64, 8, :])
        vt = keep.tile([64, EF], f32, name="vt")
        nc.sync.dma_start(out=vt[:], in_=outbounce[64:128, 8, :])

        scrU = keep.tile([64, FD], f32, name="scrU")
        uvtt = nc.vector.tensor_tensor(out=scrU[:], in0=ut[:, 0:FD],
                                       in1=vt[:, 0:FD], op=Alu.mult)
        # segment squares (gated only by big0/big1) must run before the
        # ut/vt-gated ops, or the whole DVE chain waits on the slower queue
        add_dep_helper(uvtt.ins, red_b.ins, sync=False,
                       reason="squares before stats ops")
        nc.vector.reduce_sum(out=Z[0:64, 1:2], in_=scrU[:], axis=Ax.X)
        nc.vector.tensor_tensor(out=Z[0:64, 2:3], in0=vt[:, FD + 1:FD + 2],
                                in1=ut[:, FD:FD + 1], op=Alu.mult)     # Psq*L
        nc.vector.tensor_tensor(out=Z[0:64, 3:4], in0=vt[:, FD:FD + 1],
                                in1=ut[:, FD + 1:FD + 2], op=Alu.mult)  # Pbar*Lsq
        scrF = keep.tile([1, FD], f32, name="scrF")
        nc.vector.tensor_tensor(out=scrF[:], in0=last2[:, 0:FD],
                                in1=r1[:, 0:FD], op=Alu.mult)
        nc.vector.reduce_sum(out=Z[0:1, 4:5], in_=scrF[:], axis=Ax.X)  # Fe.F

        zred = psum.tile([1, 8], f32, name="zred", tag="ps_0")
        nc.tensor.matmul(out=zred[:], lhsT=ones128[:], rhs=Z[:],
                         start=True, stop=True)
        zs = keep.tile([1, 8], f32, name="zs")
        nc.vector.tensor_copy(out=zs[:], in_=zred[:])

        # scalars: M=last2[256], a=last2[257], e=r1[256], se=r1[257]
        Mv = last2[0:1, FD:FD + 1]
        av = last2[0:1, FD + 1:FD + 2]
        ev = r1[0:1, FD:FD + 1]
        sev = r1[0:1, FD + 1:FD + 2]
        s_center = zs[0:1, 0:1]
        uv = zs[0:1, 1:2]
        psql = zs[0:1, 2:3]
        pbarlsq = zs[0:1, 3:4]
        fef = zs[0:1, 4:5]

        fin = keep.tile([1, 16], f32, name="fin")
        t_ = lambda i: fin[0:1, i:i + 1]
        # f0 = se*M ; f1 = a*e ; f2 = f0+f1
        nc.vector.tensor_tensor(out=t_(8), in0=sev, in1=Mv, op=Alu.mult)
        nc.vector.tensor_tensor(out=t_(9), in0=av, in1=ev, op=Alu.mult)
        nc.vector.tensor_tensor(out=t_(10), in0=t_(8), in1=t_(9), op=Alu.add)
        # f3 = -2*fef + f2
        nc.vector.tensor_scalar(out=t_(11), in0=fef, scalar1=-2.0,
                                scalar2=t_(10), op0=Alu.mult, op1=Alu.add)
        # f4 = f3 - psql ; f5 = f4 - pbarlsq
        nc.vector.tensor_tensor(out=t_(12), in0=t_(11), in1=psql, op=Alu.subtract)
        nc.vector.tensor_tensor(out=t_(13), in0=t_(12), in1=pbarlsq, op=Alu.subtract)
        # SD = 2*uv + f5
        nc.vector.tensor_scalar(out=t_(14), in0=uv, scalar1=2.0,
                                scalar2=t_(13), op0=Alu.mult, op1=Alu.add)
        # md = M*(M-1) ; rmd = 1/md ; div = SD*rmd*(-1/D)
        nc.vector.tensor_scalar(out=t_(15), in0=Mv, scalar1=-1.0,
                                scalar2=Mv, op0=Alu.add, op1=Alu.mult)
        nc.vector.reciprocal(t_(15), t_(15))
        nc.vector.tensor_tensor(out=t_(1), in0=t_(14), in1=t_(15), op=Alu.mult)
        nc.vector.tensor_scalar_mul(out=t_(1), in0=t_(1), scalar1=-1.0 / D)
        # tight = (a - s_center)/M
        nc.vector.tensor_tensor(out=t_(7), in0=av, in1=s_center, op=Alu.subtract)
        nc.vector.reciprocal(t_(6), Mv)
        nc.vector.tensor_tensor(out=t_(2), in0=t_(7), in1=t_(6), op=Alu.mult)
        # total = 0.1*div + 0.1*tight
        nc.vector.tensor_tensor(out=t_(0), in0=t_(1), in1=t_(2), op=Alu.add)
        nc.vector.tensor_scalar_mul(out=t_(0), in0=t_(0), scalar1=0.1)
        # debug slots
        nc.vector.tensor_copy(out=t_(3), in_=Mv)
        nc.vector.tensor_copy(out=t_(4), in_=av)
        nc.vector.tensor_copy(out=t_(5), in_=sev)

        nc.sync.dma_start(out=outd[None, :], in_=fin[0:1, 0:8])

    nc.finalize()
    return nc


def _get_compiled():
    if "nc" not in _compiled:
        _compiled["nc"] = _build_bass()
    return _compiled["nc"]


def _make_in_maps(features, targets, mask):
    features = np.ascontiguousarray(np.asarray(features, dtype=np.float32))
    targets = np.ascontiguousarray(np.asarray(targets, dtype=np.float32))
    maskf = np.asarray(mask).astype(np.float32).reshape(B, 1)
    in_maps = []
    for i in range(NCORES):
        sl = slice(i * RB, (i + 1) * RB)
        in_maps.append({
            "features": features[sl],
            "targets": targets[sl],
            "maskf": np.ascontiguousarray(maskf[sl]),
        })
    return in_maps


def kernel(features, targets, mask):
    from concourse.bass_utils import run_bass_kernel_spmd

    nc = _get_compiled()
    in_maps = _make_in_maps(features, targets, mask)
    res = run_bass_kernel_spmd(nc, in_maps, list(range(NCORES)))
    out = res.results[0]["out"]
    total = np.float32(out[0])
    diversity = np.float32(out[1])
    tightness = np.float32(out[2])
    return total, diversity, tightness



# revision 23
# speedup vs baseline: 1.2108x; 1.2108x over previous
"""Trainium2 Bass kernel for CategoricalEntropyRegLoss.

Math: both loss terms factor so the [B,B] pairwise matrices are never built.

  feat_dists = sq_j + sq_k - 2 fn_j.fn_k            (rank FD+2)
  target_dists = (E_j - P_j.LQ_k) / D               (rank DC+1)
  S = sum_{jk} m_j m_k feat_dists * target_dists    (diag is exactly 0)
    = [ se*M + a*e - 2 Fe.F - Psq.L - Pbar.Lsq + 2 <U,V> ] / D
  tightness*M = a - sum_s ||seg_sum_s||^2 / max(cnt_s,1)

Per core: one fp16 matmul  out[1154, 258] = ext_seg^T @ ext_feat with
  ext_seg  = [ onehot(code) | LQ | P | 1 | E+6.5 ]  (512 x 1154)
  ext_feat = [ m*fn | m | m*sq ]                    (512 x 258)
then exactly ONE collective: an fp16 AllReduce of the [128,10,258]
result (8 seg tiles + UV stats tile + ones/E' tile), and a fully
redundant local epilogue on every core.

Why one collective: the kernel entry barrier (present whenever a kernel
has collectives) absorbs ~40us of cross-core launch skew, then mesh
bring-up costs ~11us once and each collective costs ~10us fixed + ~10us
per one-way MB.  Any second collective (e.g. ReduceScatter + scalar
AllReduce) adds ~16us of serial fixed cost - more than the AllReduce's
extra bandwidth + the full epilogue (~6us).

Why the E+6.5 shift: e, se ~ -2.7e4 reduce in fp16 with ~3.0 abs error
(ring adds round at the running-sum magnitude), which through the ~3x
cancellation in SD costs ~3e-3 rel err.  Shifting E by its typical
value (~-6.5: two near-uniform 32-class entropies) makes the reduced
scalars O(1e2), and the shift is undone exactly in the epilogue via
  Fe = Fe' - 6.5 F,  e = e' - 6.5 M,  se = se' - 6.5 a
(the shift is exact algebra for ANY inputs; only the fp16 error size
depends on how close 6.5 is to the true mean entropy).
"""

import numpy as np

B = 4096
FD = 256
C = 32
D = 2
NSEG = C ** D          # 1024
NCORES = 8
RB = B // NCORES       # 512 rows per core
KT = RB // 128         # 4 k-chunks of 128 rows
EF = FD + 2            # 258: [mfn | m | m*sq]
NST = 2 * D * C + 2    # 130 stats columns: [lq | p | ones | E']
SEGMT = NSEG // 128    # 8 seg m-tiles
NMT = SEGMT + 2        # + UV stats tile + ones/E' tile
ESHIFT = 6.5           # typical -E of two near-uniform 32-class dists

_compiled = {}


def _build_bass():
    from contextlib import ExitStack
    import concourse.bass as bass
    import concourse.bacc as bacc
    import concourse.tile as tile
    from concourse import mybir

    from concourse.tile import add_dep_helper
    from concourse.bass import broadcast_tensor_aps

    f32 = mybir.dt.float32
    f16 = mybir.dt.float16
    Alu = mybir.AluOpType
    Act = mybir.ActivationFunctionType
    Ax = mybir.AxisListType

    nc = bacc.Bacc(num_devices=NCORES)

    feat = nc.dram_tensor("features", [RB, FD], f32, kind="ExternalInput")
    targ = nc.dram_tensor("targets", [RB, D * C], f32, kind="ExternalInput")
    maskf = nc.dram_tensor("maskf", [RB, 1], f32, kind="ExternalInput")
    outd = nc.dram_tensor("out", [8], f32, kind="ExternalOutput")

    with ExitStack() as ctx:
        tc = ctx.enter_context(tile.TileContext(nc))
        consts = ctx.enter_context(tc.tile_pool(name="consts", bufs=1))
        work = ctx.enter_context(tc.tile_pool(name="work", bufs=1))
        keep = ctx.enter_context(tc.tile_pool(name="keep", bufs=1))
        psum = ctx.enter_context(tc.tile_pool(name="psum", bufs=1, space="PSUM"))
        dram = ctx.enter_context(tc.tile_pool(name="dram", bufs=1, space="DRAM"))

        # ---------------- constants ----------------
        ones128 = consts.tile([128, 1], f32)
        nc.vector.memset(ones128[:], 1.0)

        # collective bounce buffers
        inb2 = dram.tile([128, NMT, EF], f16, name="inb2")
        outb2 = dram.tile([128, NMT, EF], f16, name="outb2",
                          addr_space="Shared")

        # ---- batched input loads spread over queues ----
        tbig = keep.tile([128, KT, D * C], f32, name="tbig")
        nc.scalar.dma_start(
            out=tbig[:], in_=targ[:, :].rearrange("(a p) f -> p a f", p=128))
        mkbig = keep.tile([128, KT, 1], f32, name="mkbig")
        nc.scalar.dma_start(
            out=mkbig[:], in_=maskf[:, :].rearrange("(a p) f -> p a f", p=128))
        # two tiles (not halves of one) so chunk reads only wait their own DMA
        xbig0 = keep.tile([128, 2, FD], f32, name="xbig0")
        nc.sync.dma_start(
            out=xbig0[:],
            in_=feat[0:256, :].rearrange("(a p) f -> p a f", p=128))
        xbig1 = keep.tile([128, 2, FD], f32, name="xbig1")
        nc.gpsimd.dma_start(
            out=xbig1[:],
            in_=feat[256:512, :].rearrange("(a p) f -> p a f", p=128))

        def xchunk(kc):
            return xbig0[:, kc, :] if kc < 2 else xbig1[:, kc - 2, :]

        # iotas after the gpsimd input DMA trigger (not needed until ~15us)
        iota1024 = consts.tile([128, NSEG], f16)
        nc.gpsimd.iota(iota1024[:], [[1, NSEG]], channel_multiplier=0,
                       allow_small_or_imprecise_dtypes=True)
        # biota[j] = 32 - j  (for first-argmax via reduce_max)
        biota = consts.tile([128, 1, 1, C], f16)
        nc.gpsimd.iota(biota[:], [[0, 1], [0, 1], [-1, C]], base=C,
                       channel_multiplier=0,
                       allow_small_or_imprecise_dtypes=True)

        # matmul operand tiles, all fp16 (PSUM accumulates fp32)
        es_oh = keep.tile([128, KT, NSEG], f16, name="esoh")
        es_st = keep.tile([128, KT, NST], f16, name="esst")
        ef = keep.tile([128, KT, EF], f16, name="ef")

        # ---- ACT phase 1: row sum-of-squares (Square table loads once) ----
        sqpack = keep.tile([128, KT], f32, name="sqpack")
        scrsq = keep.tile([128, FD], f32, name="scrsq")
        act_chain = []
        for kc in range(KT):
            act_chain.append(nc.scalar.activation(
                out=scrsq[:], in_=xchunk(kc), func=Act.Square,
                accum_out=sqpack[:, kc:kc + 1]))
        # ---- ACT phase 2: one Sqrt for all chunks ----
        normpack = keep.tile([128, KT], f32, name="normpack")
        act_chain.append(nc.scalar.sqrt(normpack[:], sqpack[:]))
        nc.vector.tensor_scalar_max(out=normpack[:], in0=normpack[:],
                                    scalar1=1e-12)
        invpack = keep.tile([128, KT], f32, name="invpack")
        nc.vector.reciprocal(invpack[:], normpack[:])
        # minv = m * inv  (fold mask into the normalization scale)
        minvpack = keep.tile([128, KT], f32, name="minvpack")
        nc.vector.tensor_tensor(out=minvpack[:], in0=invpack[:],
                                in1=mkbig[:, :, 0], op=Alu.mult)

        # ---- targets chains (DVE) ----
        t1big = keep.tile([128, KT, D * C], f32, name="t1big")
        nc.vector.tensor_scalar_add(out=t1big[:], in0=tbig[:], scalar1=1e-10)
        invsb = keep.tile([128, KT * D], f32, name="invsb")
        nc.vector.reduce_sum(
            out=invsb[:],
            in_=t1big[:].rearrange("p a (d c) -> p (a d) c", c=C),
            axis=Ax.X)
        nc.vector.reciprocal(invsb[:], invsb[:])
        # ones column early (no deps)
        nc.vector.memset(es_st[:, :, NST - 2:NST - 1], 1.0)
        # p columns (fp16 out)
        for kc in range(KT):
            for d_ in range(D):
                nc.vector.tensor_scalar_mul(
                    out=es_st[:, kc, D * C + C * d_:D * C + C * (d_ + 1)],
                    in0=t1big[:, kc, C * d_:C * (d_ + 1)],
                    scalar1=invsb[:, kc * D + d_:kc * D + d_ + 1])

        # ---- ext_feat = [x*(m*inv) | m | sq0*inv*minv] ----
        # DVE columns first (they share the ef tile with the ACT copies and
        # must not queue behind the one-hots), then the ACT feature copies
        for kc in range(KT):
            nc.vector.tensor_copy(out=ef[:, kc, FD:FD + 1],
                                  in_=mkbig[:, kc, :])
            nc.vector.tensor_scalar(out=ef[:, kc, FD + 1:FD + 2],
                                    in0=sqpack[:, kc:kc + 1],
                                    scalar1=invpack[:, kc:kc + 1],
                                    scalar2=minvpack[:, kc:kc + 1],
                                    op0=Alu.mult, op1=Alu.mult)
        copy_acts = []
        for kc in range(KT):
            copy_acts.append(nc.scalar.activation(
                out=ef[:, kc, 0:FD], in_=xchunk(kc), func=Act.Copy,
                scale=minvpack[:, kc:kc + 1]))

        # ---- batched first-argmax per (chunk,dim), then codes ----
        ptv = es_st[:, :, D * C:2 * D * C].rearrange("p a (d c) -> p a d c",
                                                     c=C)
        mx = work.tile([128, KT, D, 1], f16, name="mx", tag="mx")
        nc.vector.reduce_max(out=mx[:, :, :, 0], in_=ptv, axis=Ax.X)
        eq = work.tile([128, KT, D, C], f16, name="eq", tag="eq")
        _pa, _mb = broadcast_tensor_aps(ptv, mx[:])
        nc.vector.tensor_tensor(out=eq[:], in0=_pa, in1=_mb, op=Alu.is_equal)
        cand = work.tile([128, KT, D, C], f16, name="cand", tag="cand")
        _ea, _bb = broadcast_tensor_aps(eq[:], biota[:])
        nc.vector.tensor_tensor(out=cand[:], in0=_ea, in1=_bb,
                                op=Alu.mult)
        mq = work.tile([128, KT, D], f16, name="mq", tag="mq")
        nc.vector.reduce_max(out=mq[:], in_=cand[:], axis=Ax.X)
        cls = work.tile([128, KT, D], f16, name="cls", tag="cls")
        nc.vector.tensor_scalar(
            out=cls[:], in0=mq[:], scalar1=-1.0,
            scalar2=float(C), op0=Alu.mult, op1=Alu.add)
        code = work.tile([128, KT], f32, name="code", tag="code")
        nc.vector.scalar_tensor_tensor(
            out=code[:], in0=cls[:, :, 1], scalar=float(C), in1=cls[:, :, 0],
            op0=Alu.mult, op1=Alu.add)
        # ---- one-hot (DVE, fp16) ----
        for kc in range(KT):
            nc.vector.tensor_scalar(
                out=es_oh[:, kc, :], in0=iota1024[:],
                scalar1=code[:, kc:kc + 1],
                scalar2=None, op0=Alu.is_equal)

        # ---- ACT phase 4: Ln on the p block -> lq block ----
        ln_acts = []
        for kc in range(KT):
            ln_acts.append(nc.scalar.activation(
                out=es_st[:, kc, 0:D * C], in_=es_st[:, kc, D * C:2 * D * C],
                func=Act.Ln))

        # ---- E' column: 6.5 + sum p*lq over both dims (DVE, after Ln) ----
        escr = work.tile([128, KT], f32, name="escr", tag="escr")
        for kc in range(KT):
            scr64 = work.tile([128, D * C], f32, name=f"scr64_{kc}",
                              tag=f"s64_{kc}")
            nc.vector.tensor_tensor(
                out=scr64[:], in0=es_st[:, kc, D * C:2 * D * C],
                in1=es_st[:, kc, 0:D * C], op=Alu.mult)
            nc.vector.reduce_sum(out=escr[:, kc:kc + 1], in_=scr64[:],
                                 axis=Ax.X)
            nc.vector.tensor_scalar_add(out=es_st[:, kc, NST - 1:NST],
                                        in0=escr[:, kc:kc + 1],
                                        scalar1=ESHIFT)

        # keep ACT ops grouped by function (avoid act-table reload thrash);
        # table-less Copies before Ln so ef (gating the seg matmuls) lands
        # sooner
        act_chain = act_chain + copy_acts + ln_acts
        for a, b in zip(act_chain[1:], act_chain[:-1]):
            add_dep_helper(a.ins, b.ins, sync=False,
                           reason="act table grouping")

        # ---------------- seg m-tiles 0..7 (fp16) ----------------
        segres = keep.tile([128, SEGMT, EF], f16, name="segres")
        for mt in range(SEGMT):
            ps = psum.tile([128, EF], f32, name=f"ps_{mt}",
                           tag=f"ps_{mt % 4}")
            for kc in range(KT):
                nc.tensor.matmul(out=ps[:],
                                 lhsT=es_oh[:, kc, mt * 128:(mt + 1) * 128],
                                 rhs=ef[:, kc, :],
                                 start=(kc == 0), stop=(kc == KT - 1))
            nc.vector.tensor_copy(out=segres[:, mt, :], in_=ps[:])
            if mt == SEGMT // 2 - 1:
                nc.sync.dma_start(out=inb2[:, 0:SEGMT // 2, :],
                                  in_=segres[:, 0:SEGMT // 2, :])
            elif mt == SEGMT - 1:
                nc.scalar.dma_start(out=inb2[:, SEGMT // 2:SEGMT, :],
                                    in_=segres[:, SEGMT // 2:SEGMT, :])

        # ---------------- stats m-tile: [V(0:64); U(64:128)] ---------------
        ps8 = psum.tile([128, EF], f32, name="ps8", tag="ps_4")
        for kc in range(KT):
            nc.tensor.matmul(out=ps8[:], lhsT=es_st[:, kc, 0:128],
                             rhs=ef[:, kc, :],
                             start=(kc == 0), stop=(kc == KT - 1))
        resUV = keep.tile([128, EF], f16, name="resUV")
        nc.vector.tensor_copy(out=resUV[:], in_=ps8[:])
        nc.sync.dma_start(out=inb2[:, 8, :], in_=resUV[:])

        # mt9: [ones; E'] rows = es_st cols 128:130, zero-padded to 128 rows
        ps9 = psum.tile([2, EF], f32, name="ps9", tag="ps_5")
        for kc in range(KT):
            nc.tensor.matmul(out=ps9[:], lhsT=es_st[:, kc, 128:130],
                             rhs=ef[:, kc, :],
                             start=(kc == 0), stop=(kc == KT - 1))
        z2 = keep.tile([128, EF], f16, name="z2")
        nc.vector.memset(z2[:], 0.0)
        nc.vector.tensor_copy(out=z2[0:2, :], in_=ps9[:])
        nc.scalar.dma_start(out=inb2[:, 9, :], in_=z2[:])

        # ---------------- THE collective: one fp16 AllReduce --------------
        nc.gpsimd.collective_compute(
            "AllReduce", mybir.AluOpType.add,
            replica_groups=[list(range(NCORES))],
            ins=[inb2.opt()], outs=[outb2.opt()])

        # ---------------- redundant local epilogue ----------------
        # seg norms: sum_s ||seg_s||^2 / max(cnt_s, 1) over all 1024 rows
        segt = keep.tile([128, SEGMT, EF], f16, name="segt")
        nc.sync.dma_start(out=segt[:], in_=outb2[:, 0:SEGMT, :])
        uvt = keep.tile([64, 2, EF], f16, name="uvt")
        nc.scalar.dma_start(out=uvt[:],
                            in_=outb2[:, 8, :].rearrange("(t p) f -> p t f",
                                                         t=2))
        oet = keep.tile([1, 2, EF], f16, name="oet")
        nc.scalar.dma_start(out=oet[:], in_=outb2[0:2, 9, :][None, :, :])

        scrS = keep.tile([128, SEGMT, FD], f32, name="scrS")
        nrm = keep.tile([128, SEGMT, 4], f32, name="nrm")
        nc.vector.tensor_tensor(out=scrS[:], in0=segt[:, :, 0:FD],
                                in1=segt[:, :, 0:FD], op=Alu.mult)
        nc.vector.reduce_sum(out=nrm[:, :, 0], in_=scrS[:], axis=Ax.X)
        nc.vector.tensor_scalar_max(out=nrm[:, :, 1], in0=segt[:, :, FD],
                                    scalar1=1.0)
        nc.vector.reciprocal(nrm[:, :, 2], nrm[:, :, 1])
        nc.vector.tensor_tensor(out=nrm[:, :, 3], in0=nrm[:, :, 0],
                                in1=nrm[:, :, 2], op=Alu.mult)
        # stats: col0 = 2*<U_i,V_i>_f ; col1 = Lsq*Pbar ; col2 = Lbar*Psq
        Zst = keep.tile([64, 4], f32, name="Zst")
        scrU = keep.tile([64, FD], f32, name="scrU")
        nc.vector.scalar_tensor_tensor(out=scrU[:], in0=uvt[:, 0, 0:FD],
                                       scalar=2.0, in1=uvt[:, 1, 0:FD],
                                       op0=Alu.mult, op1=Alu.mult)
        nc.vector.reduce_sum(out=Zst[:, 0:1], in_=scrU[:], axis=Ax.X)
        nc.vector.tensor_tensor(out=Zst[:, 1:2], in0=uvt[:, 0, FD + 1:FD + 2],
                                in1=uvt[:, 1, FD:FD + 1], op=Alu.mult)
        nc.vector.tensor_tensor(out=Zst[:, 2:3], in0=uvt[:, 0, FD:FD + 1],
                                in1=uvt[:, 1, FD + 1:FD + 2], op=Alu.mult)
        # fold: col3 = col0 - col1 - col2
        nc.vector.tensor_tensor(out=Zst[:, 3:4], in0=Zst[:, 0:1],
                                in1=Zst[:, 1:2], op=Alu.subtract)
        nc.vector.tensor_tensor(out=Zst[:, 3:4], in0=Zst[:, 3:4],
                                in1=Zst[:, 2:3], op=Alu.subtract)

        # cross-partition sums via one matmul pair into one PSUM bank
        zred = psum.tile([1, 9], f32, name="zred", tag="ps_6")
        nc.tensor.matmul(out=zred[0:1, 0:8], lhsT=ones128[:],
                         rhs=nrm[:, :, 3], start=True, stop=True)
        nc.tensor.matmul(out=zred[0:1, 8:9], lhsT=ones128[0:64, :],
                         rhs=Zst[:, 3:4], start=True, stop=True)
        zs = keep.tile([1, 4], f32, name="zs")
        nc.vector.reduce_sum(out=zs[0:1, 0:1], in_=zred[0:1, 0:8], axis=Ax.X)
        nc.vector.tensor_copy(out=zs[0:1, 1:2], in_=zred[0:1, 8:9])

        # ---------------- final scalar chain ----------------
        # oet row0 = [F | M | a] ; row1 = [Fe' | e' | se']
        oef = keep.tile([1, 2, EF], f32, name="oef")
        nc.vector.tensor_copy(out=oef[:], in_=oet[:])
        Mv = oef[0:1, 0, FD:FD + 1]
        av = oef[0:1, 0, FD + 1:FD + 2]
        ev = oef[0:1, 1, FD:FD + 1]       # e'
        sev = oef[0:1, 1, FD + 1:FD + 2]  # se'

        scrF = keep.tile([1, 2, FD], f32, name="scrF")
        fin = keep.tile([1, 16], f32, name="fin")
        nc.vector.memset(fin[:], 0.0)
        t_ = lambda i: fin[0:1, i:i + 1]
        # t8 = -2*Fe'.F ; t14 = ||F||^2 (fp32 F row for both)
        nc.vector.scalar_tensor_tensor(out=scrF[0:1, 0, :],
                                       in0=oef[0:1, 1, 0:FD],
                                       scalar=-2.0, in1=oef[0:1, 0, 0:FD],
                                       op0=Alu.mult, op1=Alu.mult)
        nc.vector.reduce_sum(out=t_(8), in_=scrF[0:1, 0, :], axis=Ax.X)
        nc.vector.tensor_tensor(out=scrF[0:1, 1, :], in0=oef[0:1, 0, 0:FD],
                                in1=oef[0:1, 0, 0:FD], op=Alu.mult)
        nc.vector.reduce_sum(out=t_(14), in_=scrF[0:1, 1, :], axis=Ax.X)
        # unshift:  -2*Fe.F = t8 + 13*||F||^2
        #   se*M + a*e = se'*M + a*e' - 13*ESHIFT*a*M   (using M for a once)
        nc.vector.tensor_scalar(out=t_(15), in0=t_(14),
                                scalar1=2.0 * ESHIFT, scalar2=t_(8),
                                op0=Alu.mult, op1=Alu.add)
        # t9 = se'*M ; t10 = a*e' ; t11 = t9+t10 ; t12 = t11 - 2*ESHIFT*a*M
        nc.vector.tensor_tensor(out=t_(9), in0=sev, in1=Mv, op=Alu.mult)
        nc.vector.tensor_tensor(out=t_(10), in0=av, in1=ev, op=Alu.mult)
        nc.vector.tensor_tensor(out=t_(11), in0=t_(9), in1=t_(10), op=Alu.add)
        nc.vector.tensor_tensor(out=t_(6), in0=av, in1=Mv, op=Alu.mult)
        nc.vector.tensor_scalar(out=t_(12), in0=t_(6),
                                scalar1=-2.0 * ESHIFT, scalar2=t_(11),
                                op0=Alu.mult, op1=Alu.add)
        # SD = t12 + t15 + 2uv - psql - pbarlsq
        nc.vector.tensor_tensor(out=t_(13), in0=t_(12), in1=t_(15),
                                op=Alu.add)
        nc.vector.tensor_tensor(out=t_(13), in0=t_(13), in1=zs[0:1, 1:2],
                                op=Alu.add)
        # t7 = 1/(M*(M-1)) ; div = SD * t7 * (-1/D)
        nc.vector.tensor_scalar(out=t_(7), in0=Mv, scalar1=-1.0,
                                scalar2=Mv, op0=Alu.add, op1=Alu.mult)
        nc.vector.reciprocal(t_(7), t_(7))
        nc.vector.tensor_tensor(out=t_(1), in0=t_(13), in1=t_(7),
                                op=Alu.mult)
        nc.vector.tensor_scalar_mul(out=t_(1), in0=t_(1), scalar1=-1.0 / D)
        # tight = (a - s_center)/M
        nc.vector.tensor_tensor(out=t_(5), in0=av, in1=zs[0:1, 0:1],
                                op=Alu.subtract)
        nc.vector.reciprocal(t_(4), Mv)
        nc.vector.tensor_tensor(out=t_(2), in0=t_(5), in1=t_(4), op=Alu.mult)
        # total = 0.1*div + 0.1*tight
        nc.vector.tensor_tensor(out=t_(0), in0=t_(1), in1=t_(2), op=Alu.add)
        nc.vector.tensor_scalar_mul(out=t_(0), in0=t_(0), scalar1=0.1)
        nc.vector.tensor_copy(out=t_(3), in_=Mv)

        nc.sync.dma_start(out=outd[None, :], in_=fin[0:1, 0:8])

    nc.finalize()
    return nc


def _get_compiled():
    if "nc" not in _compiled:
        _compiled["nc"] = _build_bass()
    return _compiled["nc"]


def _make_in_maps(features, targets, mask):
    features = np.ascontiguousarray(np.asarray(features, dtype=np.float32))
    targets = np.ascontiguousarray(np.asarray(targets, dtype=np.float32))
    maskf = np.asarray(mask).astype(np.float32).reshape(B, 1)
    in_maps = []
    for i in range(NCORES):
        sl = slice(i * RB, (i + 1) * RB)
        in_maps.append({
            "features": features[sl],
            "targets": targets[sl],
            "maskf": np.ascontiguousarray(maskf[sl]),
        })
    return in_maps


def kernel(features, targets, mask):
    from concourse.bass_utils import run_bass_kernel_spmd

    nc = _get_compiled()
    in_maps = _make_in_maps(features, targets, mask)
    res = run_bass_kernel_spmd(nc, in_maps, list(range(NCORES)))
    out = res.results[0]["out"]
    total = np.float32(out[0])
    diversity = np.float32(out[1])
    tightness = np.float32(out[2])
    return total, diversity, tightness


# revision 24
# speedup vs baseline: 1.2583x; 1.0392x over previous
"""Trainium2 Bass kernel for CategoricalEntropyRegLoss.

Math: both loss terms factor so the [B,B] pairwise matrices are never built.

  feat_dists = sq_j + sq_k - 2 fn_j.fn_k            (rank FD+2)
  target_dists = (E_j - P_j.LQ_k) / D               (rank DC+1)
  S = sum_{jk} m_j m_k feat_dists * target_dists    (diag is exactly 0)
    = [ se*M + a*e - 2 Fe.F - Psq.L - Pbar.Lsq + 2 <U,V> ] / D
  tightness*M = a - sum_s ||seg_sum_s||^2 / max(cnt_s,1)

Per core: one fp16 matmul  out[1154, 258] = ext_seg^T @ ext_feat with
  ext_seg  = [ onehot(code) | LQ | P | 1 | E+6.5 ]  (512 x 1154)
  ext_feat = [ m*fn | m | m*sq ]                    (512 x 258)
then exactly ONE collective: an fp16 AllReduce of the [128,10,258]
result (8 seg tiles + UV stats tile + ones/E' tile), and a fully
redundant local epilogue on every core.

Why one collective: the kernel entry barrier (present whenever a kernel
has collectives) absorbs ~40us of cross-core launch skew, then mesh
bring-up costs ~11us once and each collective costs ~10us fixed + ~10us
per one-way MB.  Any second collective (e.g. ReduceScatter + scalar
AllReduce) adds ~16us of serial fixed cost - more than the AllReduce's
extra bandwidth + the full epilogue (~6us).

Why the E+6.5 shift: e, se ~ -2.7e4 reduce in fp16 with ~3.0 abs error
(ring adds round at the running-sum magnitude), which through the ~3x
cancellation in SD costs ~3e-3 rel err.  Shifting E by its typical
value (~-6.5: two near-uniform 32-class entropies) makes the reduced
scalars O(1e2), and the shift is undone exactly in the epilogue via
  Fe = Fe' - 6.5 F,  e = e' - 6.5 M,  se = se' - 6.5 a
(the shift is exact algebra for ANY inputs; only the fp16 error size
depends on how close 6.5 is to the true mean entropy).
"""

import numpy as np

B = 4096
FD = 256
C = 32
D = 2
NSEG = C ** D          # 1024
NCORES = 8
RB = B // NCORES       # 512 rows per core
KT = RB // 128         # 4 k-chunks of 128 rows
EF = FD + 2            # 258: [mfn | m | m*sq]
NST = 2 * D * C + 2    # 130 stats columns: [lq | p | ones | E']
SEGMT = NSEG // 128    # 8 seg m-tiles
NMT = SEGMT + 2        # + UV stats tile + ones/E' tile
ESHIFT = 6.5           # typical -E of two near-uniform 32-class dists

_compiled = {}


def _build_bass():
    from contextlib import ExitStack
    import concourse.bass as bass
    import concourse.bacc as bacc
    import concourse.tile as tile
    from concourse import mybir

    from concourse.tile import add_dep_helper
    from concourse.bass import broadcast_tensor_aps

    f32 = mybir.dt.float32
    f16 = mybir.dt.float16
    Alu = mybir.AluOpType
    Act = mybir.ActivationFunctionType
    Ax = mybir.AxisListType

    nc = bacc.Bacc(num_devices=NCORES)

    feat = nc.dram_tensor("features", [RB, FD], f32, kind="ExternalInput")
    targ = nc.dram_tensor("targets", [RB, D * C], f32, kind="ExternalInput")
    maskf = nc.dram_tensor("maskf", [RB, 1], f32, kind="ExternalInput")
    outd = nc.dram_tensor("out", [8], f32, kind="ExternalOutput")

    with ExitStack() as ctx:
        tc = ctx.enter_context(tile.TileContext(nc))
        consts = ctx.enter_context(tc.tile_pool(name="consts", bufs=1))
        work = ctx.enter_context(tc.tile_pool(name="work", bufs=1))
        keep = ctx.enter_context(tc.tile_pool(name="keep", bufs=1))
        psum = ctx.enter_context(tc.tile_pool(name="psum", bufs=1, space="PSUM"))
        dram = ctx.enter_context(tc.tile_pool(name="dram", bufs=1, space="DRAM"))

        # ---------------- constants ----------------
        ones128 = consts.tile([128, 1], f32)
        nc.vector.memset(ones128[:], 1.0)

        # collective bounce buffers
        inb2 = dram.tile([128, NMT, EF], f16, name="inb2")
        outb2 = dram.tile([128, NMT, EF], f16, name="outb2",
                          addr_space="Shared")

        # ---- batched input loads spread over queues ----
        tbig = keep.tile([128, KT, D * C], f32, name="tbig")
        nc.scalar.dma_start(
            out=tbig[:], in_=targ[:, :].rearrange("(a p) f -> p a f", p=128))
        mkbig = keep.tile([128, KT, 1], f32, name="mkbig")
        nc.scalar.dma_start(
            out=mkbig[:], in_=maskf[:, :].rearrange("(a p) f -> p a f", p=128))
        # two tiles (not halves of one) so chunk reads only wait their own DMA
        xbig0 = keep.tile([128, 2, FD], f32, name="xbig0")
        nc.sync.dma_start(
            out=xbig0[:],
            in_=feat[0:256, :].rearrange("(a p) f -> p a f", p=128))
        xbig1 = keep.tile([128, 2, FD], f32, name="xbig1")
        nc.gpsimd.dma_start(
            out=xbig1[:],
            in_=feat[256:512, :].rearrange("(a p) f -> p a f", p=128))

        def xchunk(kc):
            return xbig0[:, kc, :] if kc < 2 else xbig1[:, kc - 2, :]

        # iotas after the gpsimd input DMA trigger (not needed until ~15us)
        iota1024 = consts.tile([128, NSEG], f16)
        nc.gpsimd.iota(iota1024[:], [[1, NSEG]], channel_multiplier=0,
                       allow_small_or_imprecise_dtypes=True)
        # biota[j] = 32 - j  (for first-argmax via reduce_max)
        biota = consts.tile([128, 1, 1, C], f16)
        nc.gpsimd.iota(biota[:], [[0, 1], [0, 1], [-1, C]], base=C,
                       channel_multiplier=0,
                       allow_small_or_imprecise_dtypes=True)

        # matmul operand tiles, all fp16 (PSUM accumulates fp32)
        es_oh = keep.tile([128, KT, NSEG], f16, name="esoh")
        es_st = keep.tile([128, KT, NST], f16, name="esst")
        ef = keep.tile([128, KT, EF], f16, name="ef")

        # ---- ACT phase 1: row sum-of-squares (Square table loads once) ----
        sqpack = keep.tile([128, KT], f32, name="sqpack")
        scrsq = keep.tile([128, FD], f32, name="scrsq")
        act_chain = []
        for kc in range(KT):
            act_chain.append(nc.scalar.activation(
                out=scrsq[:], in_=xchunk(kc), func=Act.Square,
                accum_out=sqpack[:, kc:kc + 1]))
        # ---- ACT phase 2: one Sqrt for all chunks ----
        normpack = keep.tile([128, KT], f32, name="normpack")
        act_chain.append(nc.scalar.sqrt(normpack[:], sqpack[:]))
        nc.vector.tensor_scalar_max(out=normpack[:], in0=normpack[:],
                                    scalar1=1e-12)
        invpack = keep.tile([128, KT], f32, name="invpack")
        nc.vector.reciprocal(invpack[:], normpack[:])
        # minv = m * inv  (fold mask into the normalization scale)
        minvpack = keep.tile([128, KT], f32, name="minvpack")
        nc.vector.tensor_tensor(out=minvpack[:], in0=invpack[:],
                                in1=mkbig[:, :, 0], op=Alu.mult)

        # ---- targets chains (DVE) ----
        t1big = keep.tile([128, KT, D * C], f32, name="t1big")
        nc.vector.tensor_scalar_add(out=t1big[:], in0=tbig[:], scalar1=1e-10)
        invsb = keep.tile([128, KT * D], f32, name="invsb")
        nc.vector.reduce_sum(
            out=invsb[:],
            in_=t1big[:].rearrange("p a (d c) -> p (a d) c", c=C),
            axis=Ax.X)
        nc.vector.reciprocal(invsb[:], invsb[:])
        # ones column early (no deps)
        nc.vector.memset(es_st[:, :, NST - 2:NST - 1], 1.0)
        # p columns (fp16 out)
        for kc in range(KT):
            for d_ in range(D):
                nc.vector.tensor_scalar_mul(
                    out=es_st[:, kc, D * C + C * d_:D * C + C * (d_ + 1)],
                    in0=t1big[:, kc, C * d_:C * (d_ + 1)],
                    scalar1=invsb[:, kc * D + d_:kc * D + d_ + 1])

        # ---- ext_feat = [x*(m*inv) | m | sq0*inv*minv] ----
        # DVE columns first (they share the ef tile with the ACT copies and
        # must not queue behind the one-hots), then the ACT feature copies
        for kc in range(KT):
            nc.vector.tensor_copy(out=ef[:, kc, FD:FD + 1],
                                  in_=mkbig[:, kc, :])
            nc.vector.tensor_scalar(out=ef[:, kc, FD + 1:FD + 2],
                                    in0=sqpack[:, kc:kc + 1],
                                    scalar1=invpack[:, kc:kc + 1],
                                    scalar2=minvpack[:, kc:kc + 1],
                                    op0=Alu.mult, op1=Alu.mult)
        copy_acts = []
        for kc in range(KT):
            copy_acts.append(nc.scalar.activation(
                out=ef[:, kc, 0:FD], in_=xchunk(kc), func=Act.Copy,
                scale=minvpack[:, kc:kc + 1]))

        # ---- batched first-argmax per (chunk,dim), then codes ----
        ptv = es_st[:, :, D * C:2 * D * C].rearrange("p a (d c) -> p a d c",
                                                     c=C)
        mx = work.tile([128, KT, D, 1], f16, name="mx", tag="mx")
        nc.vector.reduce_max(out=mx[:, :, :, 0], in_=ptv, axis=Ax.X)
        eq = work.tile([128, KT, D, C], f16, name="eq", tag="eq")
        _pa, _mb = broadcast_tensor_aps(ptv, mx[:])
        nc.vector.tensor_tensor(out=eq[:], in0=_pa, in1=_mb, op=Alu.is_equal)
        cand = work.tile([128, KT, D, C], f16, name="cand", tag="cand")
        _ea, _bb = broadcast_tensor_aps(eq[:], biota[:])
        nc.vector.tensor_tensor(out=cand[:], in0=_ea, in1=_bb,
                                op=Alu.mult)
        mq = work.tile([128, KT, D], f16, name="mq", tag="mq")
        nc.vector.reduce_max(out=mq[:], in_=cand[:], axis=Ax.X)
        cls = work.tile([128, KT, D], f16, name="cls", tag="cls")
        nc.vector.tensor_scalar(
            out=cls[:], in0=mq[:], scalar1=-1.0,
            scalar2=float(C), op0=Alu.mult, op1=Alu.add)
        code = work.tile([128, KT], f32, name="code", tag="code")
        nc.vector.scalar_tensor_tensor(
            out=code[:], in0=cls[:, :, 1], scalar=float(C), in1=cls[:, :, 0],
            op0=Alu.mult, op1=Alu.add)
        # ---- one-hot (DVE, fp16) ----
        for kc in range(KT):
            nc.vector.tensor_scalar(
                out=es_oh[:, kc, :], in0=iota1024[:],
                scalar1=code[:, kc:kc + 1],
                scalar2=None, op0=Alu.is_equal)

        # ---- ACT phase 4: Ln on the p block -> lq block ----
        ln_acts = []
        for kc in range(KT):
            ln_acts.append(nc.scalar.activation(
                out=es_st[:, kc, 0:D * C], in_=es_st[:, kc, D * C:2 * D * C],
                func=Act.Ln))

        # ---- E' column: 6.5 + sum p*lq over both dims (DVE, after Ln) ----
        escr = work.tile([128, KT], f32, name="escr", tag="escr")
        for kc in range(KT):
            scr64 = work.tile([128, D * C], f32, name=f"scr64_{kc}",
                              tag=f"s64_{kc}")
            nc.vector.tensor_tensor(
                out=scr64[:], in0=es_st[:, kc, D * C:2 * D * C],
                in1=es_st[:, kc, 0:D * C], op=Alu.mult)
            nc.vector.reduce_sum(out=escr[:, kc:kc + 1], in_=scr64[:],
                                 axis=Ax.X)
            nc.vector.tensor_scalar_add(out=es_st[:, kc, NST - 1:NST],
                                        in0=escr[:, kc:kc + 1],
                                        scalar1=ESHIFT)

        # keep ACT ops grouped by function (avoid act-table reload thrash);
        # table-less Copies before Ln so ef (gating the seg matmuls) lands
        # sooner
        act_chain = act_chain + copy_acts + ln_acts
        for a, b in zip(act_chain[1:], act_chain[:-1]):
            add_dep_helper(a.ins, b.ins, sync=False,
                           reason="act table grouping")

        # ---------------- seg m-tiles 0..7 (fp16) ----------------
        segres = keep.tile([128, SEGMT, EF], f16, name="segres")
        for mt in range(SEGMT):
            ps = psum.tile([128, EF], f32, name=f"ps_{mt}",
                           tag=f"ps_{mt % 4}")
            for kc in range(KT):
                nc.tensor.matmul(out=ps[:],
                                 lhsT=es_oh[:, kc, mt * 128:(mt + 1) * 128],
                                 rhs=ef[:, kc, :],
                                 start=(kc == 0), stop=(kc == KT - 1))
            nc.vector.tensor_copy(out=segres[:, mt, :], in_=ps[:])
            if mt == SEGMT // 2 - 1:
                nc.sync.dma_start(out=inb2[:, 0:SEGMT // 2, :],
                                  in_=segres[:, 0:SEGMT // 2, :])
            elif mt == SEGMT - 1:
                nc.scalar.dma_start(out=inb2[:, SEGMT // 2:SEGMT, :],
                                    in_=segres[:, SEGMT // 2:SEGMT, :])

        # ---------------- stats m-tile: [V(0:64); U(64:128)] ---------------
        ps8 = psum.tile([128, EF], f32, name="ps8", tag="ps_4")
        for kc in range(KT):
            nc.tensor.matmul(out=ps8[:], lhsT=es_st[:, kc, 0:128],
                             rhs=ef[:, kc, :],
                             start=(kc == 0), stop=(kc == KT - 1))
        resUV = keep.tile([128, EF], f16, name="resUV")
        nc.vector.tensor_copy(out=resUV[:], in_=ps8[:])
        nc.sync.dma_start(out=inb2[:, 8, :], in_=resUV[:])

        # mt9: [ones; E'] rows = es_st cols 128:130, zero-padded to 128 rows
        ps9 = psum.tile([2, EF], f32, name="ps9", tag="ps_5")
        for kc in range(KT):
            nc.tensor.matmul(out=ps9[:], lhsT=es_st[:, kc, 128:130],
                             rhs=ef[:, kc, :],
                             start=(kc == 0), stop=(kc == KT - 1))
        z2 = keep.tile([128, EF], f16, name="z2")
        nc.vector.memset(z2[:], 0.0)
        nc.vector.tensor_copy(out=z2[0:2, :], in_=ps9[:])
        nc.scalar.dma_start(out=inb2[:, 9, :], in_=z2[:])

        # ---------------- THE collective: one fp16 AllReduce --------------
        nc.gpsimd.collective_compute(
            "AllReduce", mybir.AluOpType.add,
            replica_groups=[list(range(NCORES))],
            ins=[inb2.opt()], outs=[outb2.opt()])

        # ---------------- redundant local epilogue ----------------
        # seg norms: sum_s ||seg_s||^2 / max(cnt_s, 1) over all 1024 rows
        oet = keep.tile([1, 2, EF], f16, name="oet")
        nc.scalar.dma_start(out=oet[:], in_=outb2[0:2, 9, :][None, :, :])
        uvt = keep.tile([64, 2, EF], f16, name="uvt")
        nc.scalar.dma_start(out=uvt[:],
                            in_=outb2[:, 8, :].rearrange("(t p) f -> p t f",
                                                         t=2))
        segt = keep.tile([128, SEGMT, EF], f16, name="segt")
        nc.sync.dma_start(out=segt[:, 0:SEGMT // 2, :],
                          in_=outb2[:, 0:SEGMT // 2, :])
        nc.scalar.dma_start(out=segt[:, SEGMT // 2:SEGMT, :],
                            in_=outb2[:, SEGMT // 2:SEGMT, :])

        # fp16 square scratch: 16-bit in AND out doubles DVE throughput
        scrS = keep.tile([128, SEGMT, FD], f16, name="scrS")
        nrm = keep.tile([128, SEGMT, 4], f32, name="nrm")
        nc.vector.tensor_tensor(out=scrS[:, 0:SEGMT // 2, :],
                                in0=segt[:, 0:SEGMT // 2, 0:FD],
                                in1=segt[:, 0:SEGMT // 2, 0:FD], op=Alu.mult)
        nc.vector.reduce_sum(out=nrm[:, 0:SEGMT // 2, 0],
                             in_=scrS[:, 0:SEGMT // 2, :], axis=Ax.X)
        nc.vector.tensor_tensor(out=scrS[:, SEGMT // 2:, :],
                                in0=segt[:, SEGMT // 2:, 0:FD],
                                in1=segt[:, SEGMT // 2:, 0:FD], op=Alu.mult)
        nc.vector.reduce_sum(out=nrm[:, SEGMT // 2:, 0],
                             in_=scrS[:, SEGMT // 2:, :], axis=Ax.X)
        nc.vector.tensor_scalar_max(out=nrm[:, :, 1], in0=segt[:, :, FD],
                                    scalar1=1.0)
        nc.vector.reciprocal(nrm[:, :, 2], nrm[:, :, 1])
        nc.vector.tensor_tensor(out=nrm[:, :, 3], in0=nrm[:, :, 0],
                                in1=nrm[:, :, 2], op=Alu.mult)
        # stats: col0 = 2*<U_i,V_i>_f ; col1 = Lsq*Pbar ; col2 = Lbar*Psq
        Zst = keep.tile([64, 4], f32, name="Zst")
        scrU = keep.tile([64, FD], f32, name="scrU")
        nc.vector.scalar_tensor_tensor(out=scrU[:], in0=uvt[:, 0, 0:FD],
                                       scalar=2.0, in1=uvt[:, 1, 0:FD],
                                       op0=Alu.mult, op1=Alu.mult)
        nc.vector.reduce_sum(out=Zst[:, 0:1], in_=scrU[:], axis=Ax.X)
        nc.vector.tensor_tensor(out=Zst[:, 1:2], in0=uvt[:, 0, FD + 1:FD + 2],
                                in1=uvt[:, 1, FD:FD + 1], op=Alu.mult)
        nc.vector.tensor_tensor(out=Zst[:, 2:3], in0=uvt[:, 0, FD:FD + 1],
                                in1=uvt[:, 1, FD + 1:FD + 2], op=Alu.mult)
        # fold: col3 = col0 - col1 - col2
        nc.vector.tensor_tensor(out=Zst[:, 3:4], in0=Zst[:, 0:1],
                                in1=Zst[:, 1:2], op=Alu.subtract)
        nc.vector.tensor_tensor(out=Zst[:, 3:4], in0=Zst[:, 3:4],
                                in1=Zst[:, 2:3], op=Alu.subtract)

        # cross-partition sums via one matmul pair into one PSUM bank
        zred = psum.tile([1, 9], f32, name="zred", tag="ps_6")
        nc.tensor.matmul(out=zred[0:1, 0:8], lhsT=ones128[:],
                         rhs=nrm[:, :, 3], start=True, stop=True)
        nc.tensor.matmul(out=zred[0:1, 8:9], lhsT=ones128[0:64, :],
                         rhs=Zst[:, 3:4], start=True, stop=True)
        zs = keep.tile([1, 4], f32, name="zs")
        nc.vector.reduce_sum(out=zs[0:1, 0:1], in_=zred[0:1, 0:8], axis=Ax.X)
        nc.vector.tensor_copy(out=zs[0:1, 1:2], in_=zred[0:1, 8:9])

        # ---------------- final scalar chain ----------------
        # oet row0 = [F | M | a] ; row1 = [Fe' | e' | se']
        oef = keep.tile([1, 2, EF], f32, name="oef")
        nc.vector.tensor_copy(out=oef[:], in_=oet[:])
        Mv = oef[0:1, 0, FD:FD + 1]
        av = oef[0:1, 0, FD + 1:FD + 2]
        ev = oef[0:1, 1, FD:FD + 1]       # e'
        sev = oef[0:1, 1, FD + 1:FD + 2]  # se'

        scrF = keep.tile([1, 2, FD], f32, name="scrF")
        fin = keep.tile([1, 16], f32, name="fin")
        nc.vector.memset(fin[:], 0.0)
        t_ = lambda i: fin[0:1, i:i + 1]
        # t8 = -2*Fe'.F ; t14 = ||F||^2 (fp32 F row for both)
        nc.vector.scalar_tensor_tensor(out=scrF[0:1, 0, :],
                                       in0=oef[0:1, 1, 0:FD],
                                       scalar=-2.0, in1=oef[0:1, 0, 0:FD],
                                       op0=Alu.mult, op1=Alu.mult)
        nc.vector.reduce_sum(out=t_(8), in_=scrF[0:1, 0, :], axis=Ax.X)
        nc.vector.tensor_tensor(out=scrF[0:1, 1, :], in0=oef[0:1, 0, 0:FD],
                                in1=oef[0:1, 0, 0:FD], op=Alu.mult)
        nc.vector.reduce_sum(out=t_(14), in_=scrF[0:1, 1, :], axis=Ax.X)
        # unshift:  -2*Fe.F = t8 + 13*||F||^2
        #   se*M + a*e = se'*M + a*e' - 13*ESHIFT*a*M   (using M for a once)
        nc.vector.tensor_scalar(out=t_(15), in0=t_(14),
                                scalar1=2.0 * ESHIFT, scalar2=t_(8),
                                op0=Alu.mult, op1=Alu.add)
        # t9 = se'*M ; t10 = a*e' ; t11 = t9+t10 ; t12 = t11 - 2*ESHIFT*a*M
        nc.vector.tensor_tensor(out=t_(9), in0=sev, in1=Mv, op=Alu.mult)
        nc.vector.tensor_tensor(out=t_(10), in0=av, in1=ev, op=Alu.mult)
        nc.vector.tensor_tensor(out=t_(11), in0=t_(9), in1=t_(10), op=Alu.add)
        nc.vector.tensor_tensor(out=t_(6), in0=av, in1=Mv, op=Alu.mult)
        nc.vector.tensor_scalar(out=t_(12), in0=t_(6),
                                scalar1=-2.0 * ESHIFT, scalar2=t_(11),
                                op0=Alu.mult, op1=Alu.add)
        # SD = t12 + t15 + 2uv - psql - pbarlsq
        nc.vector.tensor_tensor(out=t_(13), in0=t_(12), in1=t_(15),
                                op=Alu.add)
        nc.vector.tensor_tensor(out=t_(13), in0=t_(13), in1=zs[0:1, 1:2],
                                op=Alu.add)
        # t7 = 1/(M*(M-1)) ; div = SD * t7 * (-1/D)
        nc.vector.tensor_scalar(out=t_(7), in0=Mv, scalar1=-1.0,
                                scalar2=Mv, op0=Alu.add, op1=Alu.mult)
        nc.vector.reciprocal(t_(7), t_(7))
        nc.vector.tensor_tensor(out=t_(1), in0=t_(13), in1=t_(7),
                                op=Alu.mult)
        nc.vector.tensor_scalar_mul(out=t_(1), in0=t_(1), scalar1=-1.0 / D)
        # tight = (a - s_center)/M
        nc.vector.tensor_tensor(out=t_(5), in0=av, in1=zs[0:1, 0:1],
                                op=Alu.subtract)
        nc.vector.reciprocal(t_(4), Mv)
        nc.vector.tensor_tensor(out=t_(2), in0=t_(5), in1=t_(4), op=Alu.mult)
        # total = 0.1*div + 0.1*tight
        nc.vector.tensor_tensor(out=t_(0), in0=t_(1), in1=t_(2), op=Alu.add)
        nc.vector.tensor_scalar_mul(out=t_(0), in0=t_(0), scalar1=0.1)
        nc.vector.tensor_copy(out=t_(3), in_=Mv)

        nc.sync.dma_start(out=outd[None, :], in_=fin[0:1, 0:8])

    nc.finalize()
    return nc


def _get_compiled():
    if "nc" not in _compiled:
        _compiled["nc"] = _build_bass()
    return _compiled["nc"]


def _make_in_maps(features, targets, mask):
    features = np.ascontiguousarray(np.asarray(features, dtype=np.float32))
    targets = np.ascontiguousarray(np.asarray(targets, dtype=np.float32))
    maskf = np.asarray(mask).astype(np.float32).reshape(B, 1)
    in_maps = []
    for i in range(NCORES):
        sl = slice(i * RB, (i + 1) * RB)
        in_maps.append({
            "features": features[sl],
            "targets": targets[sl],
            "maskf": np.ascontiguousarray(maskf[sl]),
        })
    return in_maps


def kernel(features, targets, mask):
    from concourse.bass_utils import run_bass_kernel_spmd

    nc = _get_compiled()
    in_maps = _make_in_maps(features, targets, mask)
    res = run_bass_kernel_spmd(nc, in_maps, list(range(NCORES)))
    out = res.results[0]["out"]
    total = np.float32(out[0])
    diversity = np.float32(out[1])
    tightness = np.float32(out[2])
    return total, diversity, tightness
